# revision 10
# baseline (speedup 1.0000x reference)
"""Trainium2 Bass kernel for nn_Attention_56538949484622.

Full attention module (QKV proj + QK-LayerNorm + RoPE + GQA causal attention
+ output proj), tensor-parallel over heads across 8 NeuronCores.

Per-core shard g (of 8): q heads 4g..4g+3, kv head g, wqkv rows for those
heads, wo rows [256g:256(g+1)].  Each core computes a partial (B*S, DIM)
output; the host sums the 8 partials (the "all-reduce after wo").

Self-contained: hardcodes all shapes from the problem spec.
"""

import numpy as np

import concourse.bass as bass
from concourse import bacc
import concourse.mybir as mybir
from concourse.tile import TileContext
from concourse.bass_utils import run_bass_kernel_spmd
from concourse.masks import make_identity

F32 = mybir.dt.float32
F32R = mybir.dt.float32r
AF = mybir.ActivationFunctionType
ALU = mybir.AluOpType

B, S, DIM = 2, 2048, 2048
NH, NKV, HD = 32, 8, 64
NCORES = 8
QH = NH // NCORES            # 4 q heads per core
TOK = B * S                  # 4096
NBB = S // 128               # 16 token blocks per batch
QF = QH * HD                 # 256
QKF = QF + HD                # 320  (q heads + k head)
FEAT = QF + 2 * HD           # 384  (q + k + v)
EPS = 1e-5
SCALE = 1.0 / 8.0            # 1/sqrt(HD)
NQC = 2                      # q chunks of 1024 per batch
QCW = S // NQC               # 1024
NSEG = QCW // 512            # 512-wide segments per q chunk


def seg_lo(kt, qc, causal):
    """First valid 512-segment (within q chunk qc) for k tile kt."""
    if not causal:
        return 0
    return max(0, (kt * 128 - qc * QCW) // 512)


def kt_list(qc, causal):
    if not causal:
        return list(range(NBB))
    return list(range(min(NBB, (qc + 1) * QCW // 128)))


def build(mask_mode: str, use_gb: bool, dbg: bool = False):
    """mask_mode: 'causal' | 'none' | 'general'."""
    causal = mask_mode == "causal"
    nc = bacc.Bacc("TRN2", target_bir_lowering=False, debug=False)

    x_d = nc.dram_tensor("x", [TOK, DIM], F32R, kind="ExternalInput").ap()
    wt_d = nc.dram_tensor("wt", [DIM, FEAT], F32R, kind="ExternalInput").ap()
    wo_d = nc.dram_tensor("wo", [QF, DIM], F32R, kind="ExternalInput").ap()
    cos_d = nc.dram_tensor("cos5", [S, QKF], F32, kind="ExternalInput").ap()
    sin_d = nc.dram_tensor("sin5", [S, QKF], F32, kind="ExternalInput").ap()
    if use_gb:
        gam_d = nc.dram_tensor("gam5", [128, QKF], F32, kind="ExternalInput").ap()
        bet_d = nc.dram_tensor("bet5", [128, QKF], F32, kind="ExternalInput").ap()
    if mask_mode == "general":
        emt_d = nc.dram_tensor("emt", [S, S], F32R, kind="ExternalInput").ap()
    out_d = nc.dram_tensor("out", [TOK, DIM], F32, kind="ExternalOutput").ap()
    if dbg:
        dbg_qkv = nc.dram_tensor("dbg_qkv", [128, FEAT], F32, kind="ExternalOutput").ap()
        dbg_qr = nc.dram_tensor("dbg_qr", [128, QKF], F32, kind="ExternalOutput").ap()
        dbg_qt01 = nc.dram_tensor("dbg_qt01", [128, S], F32, kind="ExternalOutput").ap()
        dbg_kt2 = nc.dram_tensor("dbg_kt2", [128, S], F32, kind="ExternalOutput").ap()
        dbg_vp = nc.dram_tensor("dbg_vp", [128, NBB * (HD + 1)], F32, kind="ExternalOutput").ap()
        dbg_pt = nc.dram_tensor("dbg_pt", [128, 2 * QCW], F32, kind="ExternalOutput").ap()
        dbg_at01 = nc.dram_tensor("dbg_at01", [128, S], F32, kind="ExternalOutput").ap()
        dbg_ot = nc.dram_tensor("dbg_ot", [65, 512], F32, kind="ExternalOutput").ap()
        dbg_rec = nc.dram_tensor("dbg_rec", [1, 512], F32, kind="ExternalOutput").ap()
        dbg_bc = nc.dram_tensor("dbg_bc", [64, 512], F32, kind="ExternalOutput").ap()

    with TileContext(nc) as tc:
        with (
            tc.tile_pool(name="const", bufs=1) as constp,
            tc.tile_pool(name="resid", bufs=1) as resid,
            tc.tile_pool(name="work", bufs=2) as work,
            tc.tile_pool(name="work3", bufs=3) as work3,
        ):
            # ---- constants ----
            ident = constp.tile([128, 128], F32, tag="ident")
            make_identity(nc, ident[:])
            identr = constp.tile([128, 128], F32R, tag="identr")
            nc.vector.tensor_copy(identr[:], ident[:])
            ones16 = constp.tile([128, NBB], F32, tag="ones16")
            nc.vector.memset(ones16[:], 1.0)
            epsc = constp.tile([128, 1], F32, tag="epsc")
            nc.vector.memset(epsc[:], EPS)
            wt_sb = constp.tile([128, DIM // 128, FEAT], F32R, tag="wt")
            nc.sync.dma_start(
                wt_sb[:], wt_d.rearrange("(c p) f -> p c f", p=128)
            )
            wo_sb = constp.tile([128, 2, DIM], F32R, tag="wo")
            nc.sync.dma_start(
                wo_sb[:], wo_d.rearrange("(c p) d -> p c d", p=128)
            )
            if use_gb:
                gam_sb = constp.tile([128, QKF], F32, tag="gam")
                nc.sync.dma_start(gam_sb[:], gam_d)
                bet_sb = constp.tile([128, QKF], F32, tag="bet")
                nc.sync.dma_start(bet_sb[:], bet_d)

            for b in range(B):
                # per-batch resident tiles (tags shared across batches)
                qt01 = resid.tile([128, S], F32R, tag="qt01")
                qt23 = resid.tile([128, S], F32R, tag="qt23")
                kt2 = resid.tile([128, S], F32R, tag="kt2")
                vp = resid.tile([128, NBB, HD + 1], F32R, tag="vp")
                at01 = resid.tile([128, S], F32R, tag="at01")
                at23 = resid.tile([128, S], F32R, tag="at23")
                qtp = (qt01, qt23)
                atp = (at01, at23)

                nc.vector.tensor_copy(vp[:, :, HD : HD + 1], ones16[:].unsqueeze(2))

                # ============ Phase A: QKV + LN + RoPE + transposes ============
                with (
                    tc.tile_pool(name="psA_xt", bufs=2, space="PSUM") as ps_xt,
                    tc.tile_pool(name="psA_qkv", bufs=2, space="PSUM") as ps_qkv,
                    tc.tile_pool(name="psA_qt", bufs=2, space="PSUM") as ps_qt,
                ):
                    for i in range(NBB):
                        row0 = b * S + i * 128
                        x_sb = work.tile([128, DIM], F32R, tag="x_sb")
                        nc.sync.dma_start(x_sb[:], x_d[row0 : row0 + 128, :])
                        # transpose x block -> xT tiles [d,tok], 4 per psum bank
                        xt_sb = work.tile([128, DIM // 128, 128], F32R, tag="xt_sb")
                        for c4 in range(4):
                            xt_ps = ps_xt.tile([128, 512], F32R, tag="xt_ps")
                            for j in range(4):
                                c = c4 * 4 + j
                                nc.tensor.transpose(
                                    xt_ps[:, j * 128 : (j + 1) * 128],
                                    x_sb[:, c * 128 : (c + 1) * 128],
                                    identr[:],
                                )
                            nc.scalar.copy(
                                xt_sb[:, c4 * 4 : c4 * 4 + 4, :]
                                .rearrange("p a b -> p (a b)"),
                                xt_ps[:],
                            )
                        # QKV projection: accumulate over d chunks
                        qkv_ps = ps_qkv.tile([128, FEAT], F32, tag="qkv_ps")
                        for c in range(DIM // 128):
                            nc.tensor.matmul(
                                qkv_ps[:],
                                xt_sb[:, c, :],
                                wt_sb[:, c, :],
                                start=(c == 0),
                                stop=(c == DIM // 128 - 1),
                            )
                        if dbg and b == 0 and i == 0:
                            dq = work.tile([128, FEAT], F32, tag="dbgq")
                            nc.scalar.copy(dq[:], qkv_ps[:])
                            nc.sync.dma_start(dbg_qkv, dq[:])
                        # copy q,k to sbuf fp32; v straight to V' (f32r)
                        qk = work.tile([128, QKF], F32, tag="qk")
                        nc.vector.tensor_copy(qk[:], qkv_ps[:, 0:QKF])
                        nc.vector.tensor_copy(
                            vp[:, i, 0:HD], qkv_ps[:, QKF:FEAT]
                        )
                        # LayerNorm over hd per head (5 heads: 4q + 1k)
                        st = work.tile([128, 5, 6], F32, tag="st")
                        mv = work.tile([128, 5, 2], F32, tag="mv")
                        for h in range(5):
                            nc.vector.bn_stats(
                                st[:, h, :], qk[:, h * HD : (h + 1) * HD]
                            )
                            nc.vector.bn_aggr(mv[:, h, :], st[:, h, :])
                        stdv = work.tile([128, 5], F32, tag="stdv")
                        nc.scalar.activation(
                            stdv[:], mv[:, :, 1], AF.Sqrt, bias=epsc[:]
                        )
                        rstd = work.tile([128, 5], F32, tag="rstd")
                        nc.vector.reciprocal_approx_fast(rstd[:], stdv[:])
                        qn = work.tile([128, QKF], F32, tag="qn")
                        for h in range(5):
                            sl = slice(h * HD, (h + 1) * HD)
                            nc.vector.tensor_scalar(
                                qn[:, sl],
                                qk[:, sl],
                                mv[:, h, 0:1],
                                rstd[:, h : h + 1],
                                op0=ALU.subtract,
                                op1=ALU.mult,
                            )
                        if use_gb:
                            nc.gpsimd.tensor_mul(qn[:], qn[:], gam_sb[:])
                            nc.gpsimd.tensor_add(qn[:], qn[:], bet_sb[:])
                        # RoPE
                        cosb = work.tile([128, QKF], F32, tag="cosb")
                        nc.sync.dma_start(cosb[:], cos_d[i * 128 : i * 128 + 128, :])
                        sinb = work.tile([128, QKF], F32, tag="sinb")
                        nc.sync.dma_start(sinb[:], sin_d[i * 128 : i * 128 + 128, :])
                        sw = work.tile([128, QKF], F32, tag="sw")
                        qn3 = qn[:].rearrange("p (a t) -> p a t", t=2)
                        sw3 = sw[:].rearrange("p (a t) -> p a t", t=2)
                        nc.vector.tensor_copy(sw3[:, :, 0], qn3[:, :, 1])
                        nc.vector.tensor_copy(sw3[:, :, 1], qn3[:, :, 0])
                        t1 = work.tile([128, QKF], F32, tag="t1")
                        nc.gpsimd.tensor_mul(t1[:], qn[:], cosb[:])
                        t2 = work.tile([128, QKF], F32, tag="t2")
                        nc.gpsimd.tensor_mul(t2[:], sw[:], sinb[:])
                        qr = work.tile([128, QKF], F32R, tag="qr")
                        nc.vector.tensor_add(qr[:], t1[:], t2[:])
                        if dbg and b == 0 and i == 0:
                            nc.sync.dma_start(dbg_qr, qr[:].bitcast(F32))
                        # transpose q heads + k to [hd, tok]; upper halves of
                        # the pair tiles are filled via SBUF->SBUF DMA (the
                        # only partition-crossing path).
                        for p in range(2):
                            for hh in range(2):
                                h = 2 * p + hh
                                qt_ps = ps_qt.tile(
                                    [64, 128], F32R, tag="qt_ps", bufs=4
                                )
                                nc.tensor.transpose(
                                    qt_ps[:],
                                    qr[:, h * HD : (h + 1) * HD],
                                    identr[:],
                                )
                                if hh == 0:
                                    if p == 0:
                                        nc.scalar.copy(
                                            qtp[p][0:64, i * 128 : (i + 1) * 128],
                                            qt_ps[:],
                                        )
                                    else:
                                        nc.vector.tensor_copy(
                                            qtp[p][0:64, i * 128 : (i + 1) * 128],
                                            qt_ps[:],
                                        )
                                else:
                                    stg_t = work3.tile(
                                        [64, 128], F32R, tag="stg_t"
                                    )
                                    if p == 0:
                                        nc.scalar.copy(stg_t[:], qt_ps[:])
                                    else:
                                        nc.vector.tensor_copy(stg_t[:], qt_ps[:])
                                    nc.sync.dma_start(
                                        qtp[p][64:128, i * 128 : (i + 1) * 128],
                                        stg_t[:],
                                    )
                        kt_ps = ps_qt.tile([64, 128], F32R, tag="qt_ps", bufs=4)
                        nc.tensor.transpose(kt_ps[:], qr[:, QF:QKF], identr[:])
                        nc.scalar.copy(
                            kt2[0:64, i * 128 : (i + 1) * 128], kt_ps[:]
                        )
                        nc.sync.dma_start(
                            kt2[64:128, i * 128 : (i + 1) * 128],
                            kt2[0:64, i * 128 : (i + 1) * 128],
                        )

                if dbg and b == 0:
                    nc.sync.dma_start(dbg_qt01, qt01[:].bitcast(F32))
                    nc.sync.dma_start(dbg_kt2, kt2[:].bitcast(F32))
                    nc.sync.dma_start(
                        dbg_vp, vp[:].rearrange("p a b -> p (a b)").bitcast(F32)
                    )
                # ============ Phase B: attention ============
                with (
                    tc.tile_pool(name="psB_st", bufs=1, space="PSUM") as ps_st,
                    tc.tile_pool(name="psB_o", bufs=1, space="PSUM") as ps_o,
                ):
                    for p in range(2):
                        qt = qtp[p]
                        at = atp[p]
                        for qc in range(NQC):
                            kts = kt_list(qc, causal)
                            ot = [
                                [
                                    ps_o.tile(
                                        [65, 512], F32,
                                        tag=f"ot{h}{s}", name=f"ot{h}{s}",
                                    )
                                    for s in range(NSEG)
                                ]
                                for h in range(2)
                            ]
                            for kt in kts:
                                s0 = seg_lo(kt, qc, causal)
                                qs0 = s0 * 512
                                w = QCW - qs0
                                pt = work.tile([128, 2, QCW], F32R, tag="pt")
                                sts = []
                                for h in range(2):
                                    st_ps = ps_st.tile(
                                        [128, QCW], F32, tag=f"st{h}"
                                    )
                                    sts.append(st_ps)
                                    hb = h * 64
                                    for s in range(s0, NSEG):
                                        nc.tensor.matmul(
                                            st_ps[:, s * 512 : (s + 1) * 512],
                                            kt2[hb : hb + 64, kt * 128 : (kt + 1) * 128],
                                            qt[hb : hb + 64,
                                               qc * QCW + s * 512 : qc * QCW + (s + 1) * 512],
                                            start=True,
                                            stop=True,
                                            tile_position=(hb, 0),
                                        )
                                    nc.scalar.activation(
                                        pt[:, h, qs0:QCW],
                                        st_ps[:, qs0:QCW],
                                        AF.Exp,
                                        scale=SCALE,
                                    )
                                if mask_mode == "general":
                                    emt = work.tile([128, QCW], F32R, tag="emt")
                                    nc.sync.dma_start(
                                        emt[:],
                                        emt_d[kt * 128 : (kt + 1) * 128,
                                              qc * QCW : (qc + 1) * QCW],
                                    )
                                    for h in range(2):
                                        nc.vector.tensor_mul(
                                            pt[:, h, :], pt[:, h, :], emt[:]
                                        )
                                if causal and kt * 128 >= qc * QCW:
                                    # zero strictly-above-diagonal region
                                    nc.gpsimd.affine_select(
                                        out=pt[:, :, qs0:QCW],
                                        in_=pt[:, :, qs0:QCW],
                                        compare_op=ALU.is_ge,
                                        fill=0.0,
                                        base=qc * QCW + qs0 - kt * 128,
                                        channel_multiplier=-1,
                                        pattern=[[0, 2], [1, w]],
                                    )
                                if dbg and b == 0 and p == 0 and qc == 0 and kt == 0:
                                    nc.sync.dma_start(
                                        dbg_pt,
                                        pt[:].rearrange("p a b -> p (a b)").bitcast(F32),
                                    )
                                for s in range(s0, NSEG):
                                    last = (
                                        kt
                                        == min(
                                            kts[-1],
                                            (qc * QCW + (s + 1) * 512) // 128 - 1,
                                        )
                                        if causal
                                        else kt == kts[-1]
                                    )
                                    for h in range(2):
                                        nc.tensor.matmul(
                                            ot[h][s][:],
                                            vp[:, kt, :],
                                            pt[:, h, s * 512 : (s + 1) * 512],
                                            start=(kt == 0),
                                            stop=last,
                                        )
                            # normalize and write A^T
                            for h in range(2):
                                for s in range(NSEG):
                                    col0 = qc * QCW + s * 512
                                    den = work3.tile([1, 512], F32, tag="den")
                                    nc.vector.tensor_copy(
                                        den[:], ot[h][s][64:65, :]
                                    )
                                    rec = work3.tile([1, 512], F32, tag="rec")
                                    nc.vector.reciprocal_approx_fast(
                                        rec[:], den[:]
                                    )
                                    bcst = work3.tile([64, 512], F32, tag="bcst")
                                    nc.gpsimd.partition_broadcast(bcst[:], rec[:])
                                    if (dbg and b == 0 and p == 0 and qc == 0
                                            and h == 0 and s == 0):
                                        dot = work.tile([65, 512], F32, tag="dot")
                                        nc.scalar.copy(dot[:], ot[h][s][:])
                                        nc.sync.dma_start(dbg_ot, dot[:])
                                        nc.sync.dma_start(dbg_rec, rec[:])
                                        nc.sync.dma_start(dbg_bc, bcst[:])
                                    if h == 0:
                                        nc.vector.tensor_mul(
                                            at[0:64, col0 : col0 + 512],
                                            ot[h][s][0:64, :],
                                            bcst[:],
                                        )
                                    else:
                                        stg = work3.tile([64, 512], F32R, tag="stg")
                                        nc.vector.tensor_mul(
                                            stg[:], ot[h][s][0:64, :], bcst[:]
                                        )
                                        nc.sync.dma_start(
                                            at[64:128, col0 : col0 + 512], stg[:]
                                        )

                if dbg and b == 0:
                    nc.sync.dma_start(dbg_at01, at01[:].bitcast(F32))
                # ============ Phase C: output projection ============
                with tc.tile_pool(name="psC", bufs=2, space="PSUM") as ps_c:
                    for i in range(NBB):
                        row0 = b * S + i * 128
                        for dc in range(4):
                            o_ps = ps_c.tile([128, 512], F32, tag="o_ps")
                            for p in range(2):
                                nc.tensor.matmul(
                                    o_ps[:],
                                    atp[p][:, i * 128 : (i + 1) * 128],
                                    wo_sb[:, p, dc * 512 : (dc + 1) * 512],
                                    start=(p == 0),
                                    stop=(p == 1),
                                )
                            o_sb = work.tile([128, 512], F32, tag="o_sb")
                            if (i + dc) % 2 == 0:
                                nc.scalar.copy(o_sb[:], o_ps[:])
                            else:
                                nc.vector.tensor_copy(o_sb[:], o_ps[:])
                            nc.sync.dma_start(
                                out_d[row0 : row0 + 128, dc * 512 : (dc + 1) * 512],
                                o_sb[:],
                            )

    nc.compile()
    return nc


_CACHE = {}


def _get(mask_mode, use_gb):
    key = (mask_mode, use_gb)
    if key not in _CACHE:
        _CACHE[key] = build(mask_mode, use_gb)
    return _CACHE[key]


def _analyze_mask(mask):
    if not mask.any():
        return "none"
    neg = mask[0, 1]
    causal = np.where(
        np.tril(np.ones((S, S), dtype=bool)), np.float32(0), np.float32(neg)
    )
    if neg < -1e8 and np.array_equal(mask, causal):
        return "causal"
    return "general"


def prep_inputs(x, wqkv, wo, q_gamma, q_beta, k_gamma, k_beta,
                freqs_cos, freqs_sin, mask):
    mask_mode = _analyze_mask(np.asarray(mask))
    use_gb = not (
        np.all(q_gamma == 1.0) and np.all(k_gamma == 1.0)
        and np.all(q_beta == 0.0) and np.all(k_beta == 0.0)
    )
    x2 = np.ascontiguousarray(np.asarray(x, np.float32).reshape(TOK, DIM))
    cos_e = np.repeat(np.asarray(freqs_cos, np.float32), 2, axis=1)  # [S, 64]
    sin_e = np.repeat(np.asarray(freqs_sin, np.float32), 2, axis=1)
    sin_s = sin_e.copy()
    sin_s[:, 0::2] *= -1.0
    cos5 = np.ascontiguousarray(np.tile(cos_e, (1, 5)))
    sin5 = np.ascontiguousarray(np.tile(sin_s, (1, 5)))
    in_maps = []
    for g in range(NCORES):
        wq = wqkv[g * QF : (g + 1) * QF]
        wk = wqkv[NH * HD + g * HD : NH * HD + (g + 1) * HD]
        wv = wqkv[(NH + NKV) * HD + g * HD : (NH + NKV) * HD + (g + 1) * HD]
        wt = np.ascontiguousarray(
            np.concatenate([wq, wk, wv], axis=0).T.astype(np.float32)
        )
        wog = np.ascontiguousarray(wo[g * QF : (g + 1) * QF].astype(np.float32))
        m = {"x": x2, "wt": wt, "wo": wog, "cos5": cos5, "sin5": sin5}
        if use_gb:
            g5 = np.concatenate(
                [np.tile(np.asarray(q_gamma, np.float32), QH),
                 np.asarray(k_gamma, np.float32)]
            )
            b5 = np.concatenate(
                [np.tile(np.asarray(q_beta, np.float32), QH),
                 np.asarray(k_beta, np.float32)]
            )
            m["gam5"] = np.ascontiguousarray(np.broadcast_to(g5, (128, QKF)))
            m["bet5"] = np.ascontiguousarray(np.broadcast_to(b5, (128, QKF)))
        if mask_mode == "general":
            m["emt"] = np.ascontiguousarray(
                np.exp(np.asarray(mask, np.float32).T)
            )
        in_maps.append(m)
    return mask_mode, use_gb, in_maps


def kernel(x, wqkv, wo, q_gamma, q_beta, k_gamma, k_beta,
           freqs_cos, freqs_sin, mask, _trace=False):
    mask_mode, use_gb, in_maps = prep_inputs(
        x, wqkv, wo, q_gamma, q_beta, k_gamma, k_beta,
        freqs_cos, freqs_sin, mask,
    )
    nc = _get(mask_mode, use_gb)
    res = run_bass_kernel_spmd(
        nc, in_maps, core_ids=list(range(NCORES)), trace=_trace
    )
    out = res.results[0]["out"].astype(np.float64)
    for c in range(1, NCORES):
        out += res.results[c]["out"]
    kernel.last_result = res
    return out.astype(np.float32).reshape(B, S, DIM)


# revision 11
# speedup vs baseline: 1.1116x; 1.1116x over previous
"""Trainium2 Bass kernel for nn_Attention_56538949484622.

Full attention module (QKV proj + QK-LayerNorm + RoPE + GQA causal attention
+ output proj), tensor-parallel over heads across 8 NeuronCores.

Per-core shard g (of 8): q heads 4g..4g+3, kv head g, wqkv rows for those
heads, wo rows [256g:256(g+1)].  Each core computes a partial (B*S, DIM)
output; the host sums the 8 partials (the "all-reduce after wo").

Self-contained: hardcodes all shapes from the problem spec.
"""

import numpy as np

import concourse.bass as bass
from concourse import bacc
import concourse.mybir as mybir
from concourse.tile import TileContext
from concourse.bass_utils import run_bass_kernel_spmd
from concourse.masks import make_identity

F32 = mybir.dt.float32
F32R = mybir.dt.float32r
AF = mybir.ActivationFunctionType
ALU = mybir.AluOpType

B, S, DIM = 2, 2048, 2048
NH, NKV, HD = 32, 8, 64
NCORES = 8
QH = NH // NCORES            # 4 q heads per core
TOK = B * S                  # 4096
NBB = S // 128               # 16 token blocks per batch
QF = QH * HD                 # 256
QKF = QF + HD                # 320  (q heads + k head)
FEAT = QF + 2 * HD           # 384  (q + k + v)
EPS = 1e-5
SCALE = 1.0 / 8.0            # 1/sqrt(HD)
NQC = 2                      # q chunks of 1024 per batch
QCW = S // NQC               # 1024
NSEG = QCW // 512            # 512-wide segments per q chunk


def seg_lo(kt, qc, causal):
    """First valid 512-segment (within q chunk qc) for k tile kt."""
    if not causal:
        return 0
    return max(0, (kt * 128 - qc * QCW) // 512)


def kt_list(qc, causal):
    if not causal:
        return list(range(NBB))
    return list(range(min(NBB, (qc + 1) * QCW // 128)))


def build(mask_mode: str, use_gb: bool, dbg: bool = False):
    """mask_mode: 'causal' | 'none' | 'general'."""
    causal = mask_mode == "causal"
    nc = bacc.Bacc("TRN2", target_bir_lowering=False, debug=False)

    x_d = nc.dram_tensor("x", [TOK, DIM], F32R, kind="ExternalInput").ap()
    wt_d = nc.dram_tensor("wt", [DIM, FEAT], F32R, kind="ExternalInput").ap()
    wo_d = nc.dram_tensor("wo", [QF, DIM], F32R, kind="ExternalInput").ap()
    cos_d = nc.dram_tensor("cos5", [S, QKF], F32, kind="ExternalInput").ap()
    sin_d = nc.dram_tensor("sin5", [S, QKF], F32, kind="ExternalInput").ap()
    if use_gb:
        gam_d = nc.dram_tensor("gam5", [128, QKF], F32, kind="ExternalInput").ap()
        bet_d = nc.dram_tensor("bet5", [128, QKF], F32, kind="ExternalInput").ap()
    if mask_mode == "general":
        emt_d = nc.dram_tensor("emt", [S, S], F32R, kind="ExternalInput").ap()
    out_d = nc.dram_tensor("out", [TOK, DIM], F32, kind="ExternalOutput").ap()
    if dbg:
        dbg_qkv = nc.dram_tensor("dbg_qkv", [128, FEAT], F32, kind="ExternalOutput").ap()
        dbg_qr = nc.dram_tensor("dbg_qr", [128, QKF], F32, kind="ExternalOutput").ap()
        dbg_qt01 = nc.dram_tensor("dbg_qt01", [128, S], F32, kind="ExternalOutput").ap()
        dbg_kt2 = nc.dram_tensor("dbg_kt2", [128, S], F32, kind="ExternalOutput").ap()
        dbg_vp = nc.dram_tensor("dbg_vp", [128, NBB * (HD + 1)], F32, kind="ExternalOutput").ap()
        dbg_pt = nc.dram_tensor("dbg_pt", [128, 2 * QCW], F32, kind="ExternalOutput").ap()
        dbg_at01 = nc.dram_tensor("dbg_at01", [128, S], F32, kind="ExternalOutput").ap()
        dbg_ot = nc.dram_tensor("dbg_ot", [65, 512], F32, kind="ExternalOutput").ap()
        dbg_rec = nc.dram_tensor("dbg_rec", [1, 512], F32, kind="ExternalOutput").ap()
        dbg_bc = nc.dram_tensor("dbg_bc", [64, 512], F32, kind="ExternalOutput").ap()

    with TileContext(nc) as tc:
        with (
            tc.tile_pool(name="const", bufs=1) as constp,
            tc.tile_pool(name="resid", bufs=1) as resid,
            tc.tile_pool(name="work", bufs=2) as work,
            tc.tile_pool(name="work3", bufs=3) as work3,
        ):
            # ---- constants ----
            ident = constp.tile([128, 128], F32, tag="ident")
            make_identity(nc, ident[:])
            identr = constp.tile([128, 128], F32R, tag="identr")
            nc.vector.tensor_copy(identr[:], ident[:])
            ones16 = constp.tile([128, NBB], F32, tag="ones16")
            nc.vector.memset(ones16[:], 1.0)
            epsc = constp.tile([128, 1], F32, tag="epsc")
            nc.vector.memset(epsc[:], EPS)
            BF16 = mybir.dt.bfloat16
            atri = constp.tile([128, 128], BF16, tag="atri")
            nc.vector.memset(atri[:], 1.0)
            nc.gpsimd.affine_select(
                out=atri[:], in_=atri[:], compare_op=ALU.is_ge,
                fill=0.0, base=-1, channel_multiplier=-1, pattern=[[1, 128]],
            )
            negi = constp.tile([128, 128], BF16, tag="negi")
            nc.scalar.activation(negi[:], ident[:], AF.Copy, scale=-1.0e9)
            wt_sb = constp.tile([128, DIM // 128, FEAT], F32R, tag="wt")
            nc.sync.dma_start(
                wt_sb[:], wt_d.rearrange("(c p) f -> p c f", p=128)
            )
            wo_sb = constp.tile([128, 2, DIM], F32R, tag="wo")
            nc.sync.dma_start(
                wo_sb[:], wo_d.rearrange("(c p) d -> p c d", p=128)
            )
            if use_gb:
                gam_sb = constp.tile([128, QKF], F32, tag="gam")
                nc.sync.dma_start(gam_sb[:], gam_d)
                bet_sb = constp.tile([128, QKF], F32, tag="bet")
                nc.sync.dma_start(bet_sb[:], bet_d)

            for b in range(B):
                # per-batch resident tiles (tags shared across batches)
                qt01 = resid.tile([128, S], F32R, tag="qt01")
                qt23 = resid.tile([128, S], F32R, tag="qt23")
                kt2 = resid.tile([128, S], F32R, tag="kt2")
                vp = resid.tile([128, NBB, HD + 1], F32R, tag="vp")
                at01 = resid.tile([128, S], F32R, tag="at01")
                at23 = resid.tile([128, S], F32R, tag="at23")
                qtp = (qt01, qt23)
                atp = (at01, at23)

                nc.vector.tensor_copy(vp[:, :, HD : HD + 1], ones16[:].unsqueeze(2))

                # ============ Phase A: QKV + LN + RoPE + transposes ============
                with (
                    tc.tile_pool(name="psA_xt", bufs=2, space="PSUM") as ps_xt,
                    tc.tile_pool(name="psA_qkv", bufs=2, space="PSUM") as ps_qkv,
                    tc.tile_pool(name="psA_qt", bufs=2, space="PSUM") as ps_qt,
                ):
                    for i in range(NBB):
                        row0 = b * S + i * 128
                        x_sb = work.tile([128, DIM], F32R, tag="x_sb")
                        nc.sync.dma_start(x_sb[:], x_d[row0 : row0 + 128, :])
                        # transpose x block -> xT tiles [d,tok], 4 per psum bank
                        xt_sb = work.tile([128, DIM // 128, 128], F32R, tag="xt_sb")
                        for c4 in range(4):
                            xt_ps = ps_xt.tile([128, 512], F32R, tag="xt_ps")
                            for j in range(4):
                                c = c4 * 4 + j
                                nc.tensor.transpose(
                                    xt_ps[:, j * 128 : (j + 1) * 128],
                                    x_sb[:, c * 128 : (c + 1) * 128],
                                    identr[:],
                                )
                            nc.scalar.copy(
                                xt_sb[:, c4 * 4 : c4 * 4 + 4, :]
                                .rearrange("p a b -> p (a b)"),
                                xt_ps[:],
                            )
                        # QKV projection: accumulate over d chunks
                        qkv_ps = ps_qkv.tile([128, FEAT], F32, tag="qkv_ps")
                        for c in range(DIM // 128):
                            nc.tensor.matmul(
                                qkv_ps[:],
                                xt_sb[:, c, :],
                                wt_sb[:, c, :],
                                start=(c == 0),
                                stop=(c == DIM // 128 - 1),
                            )
                        if dbg and b == 0 and i == 0:
                            dq = work.tile([128, FEAT], F32, tag="dbgq")
                            nc.scalar.copy(dq[:], qkv_ps[:])
                            nc.sync.dma_start(dbg_qkv, dq[:])
                        # copy q,k to sbuf fp32; v straight to V' (f32r)
                        qk = work.tile([128, QKF], F32, tag="qk")
                        nc.vector.tensor_copy(qk[:], qkv_ps[:, 0:QKF])
                        nc.vector.tensor_copy(
                            vp[:, i, 0:HD], qkv_ps[:, QKF:FEAT]
                        )
                        # LayerNorm over hd per head (5 heads: 4q + 1k)
                        st = work.tile([128, 5, 6], F32, tag="st")
                        mv = work.tile([128, 5, 2], F32, tag="mv")
                        for h in range(5):
                            nc.vector.bn_stats(
                                st[:, h, :], qk[:, h * HD : (h + 1) * HD]
                            )
                            nc.vector.bn_aggr(mv[:, h, :], st[:, h, :])
                        stdv = work.tile([128, 5], F32, tag="stdv")
                        nc.scalar.activation(
                            stdv[:], mv[:, :, 1], AF.Sqrt, bias=epsc[:]
                        )
                        rstd = work.tile([128, 5], F32, tag="rstd")
                        nc.vector.reciprocal_approx_fast(rstd[:], stdv[:])
                        qn = work.tile([128, QKF], F32, tag="qn")
                        for h in range(5):
                            sl = slice(h * HD, (h + 1) * HD)
                            nc.vector.tensor_scalar(
                                qn[:, sl],
                                qk[:, sl],
                                mv[:, h, 0:1],
                                rstd[:, h : h + 1],
                                op0=ALU.subtract,
                                op1=ALU.mult,
                            )
                        if use_gb:
                            nc.gpsimd.tensor_mul(qn[:], qn[:], gam_sb[:])
                            nc.gpsimd.tensor_add(qn[:], qn[:], bet_sb[:])
                        # RoPE
                        cosb = work.tile([128, QKF], F32, tag="cosb")
                        nc.sync.dma_start(cosb[:], cos_d[i * 128 : i * 128 + 128, :])
                        sinb = work.tile([128, QKF], F32, tag="sinb")
                        nc.sync.dma_start(sinb[:], sin_d[i * 128 : i * 128 + 128, :])
                        sw = work.tile([128, QKF], F32, tag="sw")
                        qn3 = qn[:].rearrange("p (a t) -> p a t", t=2)
                        sw3 = sw[:].rearrange("p (a t) -> p a t", t=2)
                        nc.vector.tensor_copy(sw3[:, :, 0], qn3[:, :, 1])
                        nc.vector.tensor_copy(sw3[:, :, 1], qn3[:, :, 0])
                        t1 = work.tile([128, QKF], F32, tag="t1")
                        nc.gpsimd.tensor_mul(t1[:], qn[:], cosb[:])
                        t2 = work.tile([128, QKF], F32, tag="t2")
                        nc.gpsimd.tensor_mul(t2[:], sw[:], sinb[:])
                        qr = work.tile([128, QKF], F32R, tag="qr")
                        nc.vector.tensor_add(qr[:], t1[:], t2[:])
                        if dbg and b == 0 and i == 0:
                            nc.sync.dma_start(dbg_qr, qr[:].bitcast(F32))
                        # transpose q heads + k to [hd, tok]; upper halves of
                        # the pair tiles are filled via SBUF->SBUF DMA (the
                        # only partition-crossing path).
                        for p in range(2):
                            for hh in range(2):
                                h = 2 * p + hh
                                qt_ps = ps_qt.tile(
                                    [64, 128], F32R, tag="qt_ps", bufs=4
                                )
                                nc.tensor.transpose(
                                    qt_ps[:],
                                    qr[:, h * HD : (h + 1) * HD],
                                    identr[:],
                                )
                                if hh == 0:
                                    if p == 0:
                                        nc.scalar.copy(
                                            qtp[p][0:64, i * 128 : (i + 1) * 128],
                                            qt_ps[:],
                                        )
                                    else:
                                        nc.vector.tensor_copy(
                                            qtp[p][0:64, i * 128 : (i + 1) * 128],
                                            qt_ps[:],
                                        )
                                else:
                                    stg_t = work3.tile(
                                        [64, 128], F32R, tag="stg_t"
                                    )
                                    if p == 0:
                                        nc.scalar.copy(stg_t[:], qt_ps[:])
                                    else:
                                        nc.vector.tensor_copy(stg_t[:], qt_ps[:])
                                    nc.sync.dma_start(
                                        qtp[p][64:128, i * 128 : (i + 1) * 128],
                                        stg_t[:],
                                    )
                        kt_ps = ps_qt.tile([64, 128], F32R, tag="qt_ps", bufs=4)
                        nc.tensor.transpose(kt_ps[:], qr[:, QF:QKF], identr[:])
                        nc.scalar.copy(
                            kt2[0:64, i * 128 : (i + 1) * 128], kt_ps[:]
                        )
                        nc.sync.dma_start(
                            kt2[64:128, i * 128 : (i + 1) * 128],
                            kt2[0:64, i * 128 : (i + 1) * 128],
                        )

                if dbg and b == 0:
                    nc.sync.dma_start(dbg_qt01, qt01[:].bitcast(F32))
                    nc.sync.dma_start(dbg_kt2, kt2[:].bitcast(F32))
                    nc.sync.dma_start(
                        dbg_vp, vp[:].rearrange("p a b -> p (a b)").bitcast(F32)
                    )
                # ============ Phase B: attention ============
                with (
                    tc.tile_pool(name="psB_st", bufs=1, space="PSUM") as ps_st,
                    tc.tile_pool(name="psB_o", bufs=1, space="PSUM") as ps_o,
                ):
                    for p in range(2):
                        qt = qtp[p]
                        at = atp[p]
                        for qc in range(NQC):
                            kts = kt_list(qc, causal)
                            ot = [
                                [
                                    ps_o.tile(
                                        [65, 512], F32,
                                        tag=f"ot{h}{s}", name=f"ot{h}{s}",
                                    )
                                    for s in range(NSEG)
                                ]
                                for h in range(2)
                            ]
                            for kt in kts:
                                s0 = seg_lo(kt, qc, causal)
                                qs0 = s0 * 512
                                r = kt * 128 - qc * QCW
                                diag = causal and r >= 0
                                e0 = r if diag else qs0
                                pt = work.tile([128, 2, QCW], F32R, tag="pt")
                                if diag and e0 > qs0:
                                    # zero the fully-masked left columns
                                    nc.gpsimd.memset(
                                        pt[:, :, qs0:e0].bitcast(F32), 0.0
                                    )
                                for h in range(2):
                                    st_ps = ps_st.tile(
                                        [128, QCW], F32, tag=f"st{h}"
                                    )
                                    hb = h * 64
                                    if diag:
                                        # preload -1e9 above-diagonal into the
                                        # diag block; S^T accumulates on top
                                        nc.tensor.matmul(
                                            st_ps[:, r : r + 128],
                                            atri[:],
                                            negi[:],
                                            start=True,
                                            stop=False,
                                            skip_group_check=True,
                                        )
                                    for s in range(s0, NSEG):
                                        nc.tensor.matmul(
                                            st_ps[:, s * 512 : (s + 1) * 512],
                                            kt2[hb : hb + 64, kt * 128 : (kt + 1) * 128],
                                            qt[hb : hb + 64,
                                               qc * QCW + s * 512 : qc * QCW + (s + 1) * 512],
                                            start=not (diag and s == s0),
                                            stop=True,
                                            tile_position=(hb, 0),
                                            skip_group_check=True,
                                        )
                                    nc.scalar.activation(
                                        pt[:, h, e0:QCW],
                                        st_ps[:, e0:QCW],
                                        AF.Exp,
                                        scale=SCALE,
                                    )
                                if mask_mode == "general":
                                    emt = work.tile([128, QCW], F32R, tag="emt")
                                    nc.sync.dma_start(
                                        emt[:],
                                        emt_d[kt * 128 : (kt + 1) * 128,
                                              qc * QCW : (qc + 1) * QCW],
                                    )
                                    for h in range(2):
                                        nc.vector.tensor_mul(
                                            pt[:, h, :], pt[:, h, :], emt[:]
                                        )
                                if dbg and b == 0 and p == 0 and qc == 0 and kt == 0:
                                    nc.sync.dma_start(
                                        dbg_pt,
                                        pt[:].rearrange("p a b -> p (a b)").bitcast(F32),
                                    )
                                for s in range(s0, NSEG):
                                    last = (
                                        kt
                                        == min(
                                            kts[-1],
                                            (qc * QCW + (s + 1) * 512) // 128 - 1,
                                        )
                                        if causal
                                        else kt == kts[-1]
                                    )
                                    for h in range(2):
                                        nc.tensor.matmul(
                                            ot[h][s][:],
                                            vp[:, kt, :],
                                            pt[:, h, s * 512 : (s + 1) * 512],
                                            start=(kt == 0),
                                            stop=last,
                                        )
                            # normalize and write A^T
                            for h in range(2):
                                for s in range(NSEG):
                                    col0 = qc * QCW + s * 512
                                    den = work3.tile([1, 512], F32, tag="den")
                                    nc.vector.tensor_copy(
                                        den[:], ot[h][s][64:65, :]
                                    )
                                    rec = work3.tile([1, 512], F32, tag="rec")
                                    nc.vector.reciprocal_approx_fast(
                                        rec[:], den[:]
                                    )
                                    bcst = work3.tile([64, 512], F32, tag="bcst")
                                    nc.gpsimd.partition_broadcast(bcst[:], rec[:])
                                    if (dbg and b == 0 and p == 0 and qc == 0
                                            and h == 0 and s == 0):
                                        dot = work.tile([65, 512], F32, tag="dot")
                                        nc.scalar.copy(dot[:], ot[h][s][:])
                                        nc.sync.dma_start(dbg_ot, dot[:])
                                        nc.sync.dma_start(dbg_rec, rec[:])
                                        nc.sync.dma_start(dbg_bc, bcst[:])
                                    if h == 0:
                                        nc.vector.tensor_mul(
                                            at[0:64, col0 : col0 + 512],
                                            ot[h][s][0:64, :],
                                            bcst[:],
                                        )
                                    else:
                                        stg = work3.tile([64, 512], F32R, tag="stg")
                                        nc.vector.tensor_mul(
                                            stg[:], ot[h][s][0:64, :], bcst[:]
                                        )
                                        nc.sync.dma_start(
                                            at[64:128, col0 : col0 + 512], stg[:]
                                        )

                if dbg and b == 0:
                    nc.sync.dma_start(dbg_at01, at01[:].bitcast(F32))
                # ============ Phase C: output projection ============
                with tc.tile_pool(name="psC", bufs=3, space="PSUM") as ps_c:
                    for i in range(NBB):
                        row0 = b * S + i * 128
                        for dc in range(2):
                            o_ps = ps_c.tile([128, 1024], F32, tag="o_ps")
                            for dd in range(2):
                                for p in range(2):
                                    nc.tensor.matmul(
                                        o_ps[:, dd * 512 : (dd + 1) * 512],
                                        atp[p][:, i * 128 : (i + 1) * 128],
                                        wo_sb[:, p,
                                              (dc * 2 + dd) * 512 : (dc * 2 + dd + 1) * 512],
                                        start=(p == 0),
                                        stop=(p == 1),
                                    )
                            o_sb = work3.tile([128, 1024], F32, tag="o_sb")
                            if (i + dc) % 2 == 0:
                                nc.scalar.copy(o_sb[:], o_ps[:])
                            else:
                                nc.vector.tensor_copy(o_sb[:], o_ps[:])
                            nc.sync.dma_start(
                                out_d[row0 : row0 + 128,
                                      dc * 1024 : (dc + 1) * 1024],
                                o_sb[:],
                            )

    nc.compile()
    return nc


_CACHE = {}


def _get(mask_mode, use_gb):
    key = (mask_mode, use_gb)
    if key not in _CACHE:
        _CACHE[key] = build(mask_mode, use_gb)
    return _CACHE[key]


def _analyze_mask(mask):
    if not mask.any():
        return "none"
    neg = mask[0, 1]
    causal = np.where(
        np.tril(np.ones((S, S), dtype=bool)), np.float32(0), np.float32(neg)
    )
    if neg < -1e8 and np.array_equal(mask, causal):
        return "causal"
    return "general"


def prep_inputs(x, wqkv, wo, q_gamma, q_beta, k_gamma, k_beta,
                freqs_cos, freqs_sin, mask):
    mask_mode = _analyze_mask(np.asarray(mask))
    use_gb = not (
        np.all(q_gamma == 1.0) and np.all(k_gamma == 1.0)
        and np.all(q_beta == 0.0) and np.all(k_beta == 0.0)
    )
    x2 = np.ascontiguousarray(np.asarray(x, np.float32).reshape(TOK, DIM))
    cos_e = np.repeat(np.asarray(freqs_cos, np.float32), 2, axis=1)  # [S, 64]
    sin_e = np.repeat(np.asarray(freqs_sin, np.float32), 2, axis=1)
    sin_s = sin_e.copy()
    sin_s[:, 0::2] *= -1.0
    cos5 = np.ascontiguousarray(np.tile(cos_e, (1, 5)))
    sin5 = np.ascontiguousarray(np.tile(sin_s, (1, 5)))
    in_maps = []
    for g in range(NCORES):
        wq = wqkv[g * QF : (g + 1) * QF]
        wk = wqkv[NH * HD + g * HD : NH * HD + (g + 1) * HD]
        wv = wqkv[(NH + NKV) * HD + g * HD : (NH + NKV) * HD + (g + 1) * HD]
        wt = np.ascontiguousarray(
            np.concatenate([wq, wk, wv], axis=0).T.astype(np.float32)
        )
        wog = np.ascontiguousarray(wo[g * QF : (g + 1) * QF].astype(np.float32))
        m = {"x": x2, "wt": wt, "wo": wog, "cos5": cos5, "sin5": sin5}
        if use_gb:
            g5 = np.concatenate(
                [np.tile(np.asarray(q_gamma, np.float32), QH),
                 np.asarray(k_gamma, np.float32)]
            )
            b5 = np.concatenate(
                [np.tile(np.asarray(q_beta, np.float32), QH),
                 np.asarray(k_beta, np.float32)]
            )
            m["gam5"] = np.ascontiguousarray(np.broadcast_to(g5, (128, QKF)))
            m["bet5"] = np.ascontiguousarray(np.broadcast_to(b5, (128, QKF)))
        if mask_mode == "general":
            m["emt"] = np.ascontiguousarray(
                np.exp(np.asarray(mask, np.float32).T)
            )
        in_maps.append(m)
    return mask_mode, use_gb, in_maps


def kernel(x, wqkv, wo, q_gamma, q_beta, k_gamma, k_beta,
           freqs_cos, freqs_sin, mask, _trace=False):
    mask_mode, use_gb, in_maps = prep_inputs(
        x, wqkv, wo, q_gamma, q_beta, k_gamma, k_beta,
        freqs_cos, freqs_sin, mask,
    )
    nc = _get(mask_mode, use_gb)
    res = run_bass_kernel_spmd(
        nc, in_maps, core_ids=list(range(NCORES)), trace=_trace
    )
    out = res.results[0]["out"].astype(np.float64)
    for c in range(1, NCORES):
        out += res.results[c]["out"]
    kernel.last_result = res
    return out.astype(np.float32).reshape(B, S, DIM)


# revision 12
# speedup vs baseline: 1.3443x; 1.2094x over previous
"""Trainium2 Bass kernel for nn_Attention_56538949484622.

Full attention module (QKV proj + QK-LayerNorm + RoPE + GQA causal attention
+ output proj), tensor-parallel over heads across 8 NeuronCores.

Per-core shard g (of 8): q heads 4g..4g+3, kv head g, wqkv rows for those
heads, wo rows [256g:256(g+1)].  Each core computes a partial (B*S, DIM)
output; the host sums the 8 partials (the "all-reduce after wo").

Self-contained: hardcodes all shapes from the problem spec.
"""

import numpy as np

import concourse.bass as bass
from concourse import bacc
import concourse.mybir as mybir
from concourse.tile import TileContext
from concourse.bass_utils import run_bass_kernel_spmd
from concourse.masks import make_identity

F32 = mybir.dt.float32
F32R = mybir.dt.float32r
AF = mybir.ActivationFunctionType
ALU = mybir.AluOpType

B, S, DIM = 2, 2048, 2048
NH, NKV, HD = 32, 8, 64
NCORES = 8
QH = NH // NCORES            # 4 q heads per core
TOK = B * S                  # 4096
NBB = S // 128               # 16 token blocks per batch
QF = QH * HD                 # 256
QKF = QF + HD                # 320  (q heads + k head)
FEAT = QF + 2 * HD           # 384  (q + k + v)
EPS = 1e-5
SCALE = 1.0 / 8.0            # 1/sqrt(HD)
NQC = 2                      # q chunks of 1024 per batch
QCW = S // NQC               # 1024
NSEG = QCW // 512            # 512-wide segments per q chunk


def seg_lo(kt, qc, causal):
    """First valid 512-segment (within q chunk qc) for k tile kt."""
    if not causal:
        return 0
    return max(0, (kt * 128 - qc * QCW) // 512)


def kt_list(qc, causal):
    if not causal:
        return list(range(NBB))
    return list(range(min(NBB, (qc + 1) * QCW // 128)))


def build(mask_mode: str, use_gb: bool, dbg: bool = False):
    """mask_mode: 'causal' | 'none' | 'general'."""
    causal = mask_mode == "causal"
    nc = bacc.Bacc("TRN2", target_bir_lowering=False, debug=False)

    x_d = nc.dram_tensor("x", [TOK, DIM], F32R, kind="ExternalInput").ap()
    wt_d = nc.dram_tensor("wt", [DIM, FEAT], F32R, kind="ExternalInput").ap()
    wo_d = nc.dram_tensor("wo", [QF, DIM], F32R, kind="ExternalInput").ap()
    cos_d = nc.dram_tensor("cos5", [S, QKF], F32, kind="ExternalInput").ap()
    sin_d = nc.dram_tensor("sin5", [S, QKF], F32, kind="ExternalInput").ap()
    if use_gb:
        gam_d = nc.dram_tensor("gam5", [128, QKF], F32, kind="ExternalInput").ap()
        bet_d = nc.dram_tensor("bet5", [128, QKF], F32, kind="ExternalInput").ap()
    if mask_mode == "general":
        emt_d = nc.dram_tensor("emt", [S, S], F32R, kind="ExternalInput").ap()
    out_d = nc.dram_tensor("out", [TOK, DIM], F32, kind="ExternalOutput").ap()
    if dbg:
        dbg_qkv = nc.dram_tensor("dbg_qkv", [128, FEAT], F32, kind="ExternalOutput").ap()
        dbg_qr = nc.dram_tensor("dbg_qr", [128, QKF], F32, kind="ExternalOutput").ap()
        dbg_qt01 = nc.dram_tensor("dbg_qt01", [128, S], F32, kind="ExternalOutput").ap()
        dbg_kt2 = nc.dram_tensor("dbg_kt2", [128, S], F32, kind="ExternalOutput").ap()
        dbg_vp = nc.dram_tensor("dbg_vp", [128, NBB * (HD + 1)], F32, kind="ExternalOutput").ap()
        dbg_pt = nc.dram_tensor("dbg_pt", [128, 2 * QCW], F32, kind="ExternalOutput").ap()
        dbg_at01 = nc.dram_tensor("dbg_at01", [128, S], F32, kind="ExternalOutput").ap()
        dbg_ot = nc.dram_tensor("dbg_ot", [65, 512], F32, kind="ExternalOutput").ap()
        dbg_rec = nc.dram_tensor("dbg_rec", [1, 512], F32, kind="ExternalOutput").ap()
        dbg_bc = nc.dram_tensor("dbg_bc", [64, 512], F32, kind="ExternalOutput").ap()

    with TileContext(nc) as tc:
        with (
            tc.tile_pool(name="const", bufs=1) as constp,
            tc.tile_pool(name="resid", bufs=1) as resid,
            tc.tile_pool(name="work", bufs=2) as work,
            tc.tile_pool(name="work3", bufs=3) as work3,
        ):
            # ---- constants ----
            ident = constp.tile([128, 128], F32, tag="ident")
            make_identity(nc, ident[:])
            identr = constp.tile([128, 128], F32R, tag="identr")
            nc.vector.tensor_copy(identr[:], ident[:])
            ones16 = constp.tile([128, NBB], F32, tag="ones16")
            nc.vector.memset(ones16[:], 1.0)
            epsc = constp.tile([128, 1], F32, tag="epsc")
            nc.vector.memset(epsc[:], EPS)
            BF16 = mybir.dt.bfloat16
            atri = constp.tile([128, 128], BF16, tag="atri")
            nc.vector.memset(atri[:], 1.0)
            nc.gpsimd.affine_select(
                out=atri[:], in_=atri[:], compare_op=ALU.is_ge,
                fill=0.0, base=-1, channel_multiplier=-1, pattern=[[1, 128]],
            )
            negi = constp.tile([128, 128], BF16, tag="negi")
            nc.scalar.activation(negi[:], ident[:], AF.Copy, scale=-1.0e9)
            wt_sb = constp.tile([128, DIM // 128, FEAT], F32R, tag="wt")
            nc.sync.dma_start(
                wt_sb[:], wt_d.rearrange("(c p) f -> p c f", p=128)
            )
            wo_sb = constp.tile([128, 2, DIM], F32R, tag="wo")
            nc.sync.dma_start(
                wo_sb[:], wo_d.rearrange("(c p) d -> p c d", p=128)
            )
            if use_gb:
                gam_sb = constp.tile([128, QKF], F32, tag="gam")
                nc.sync.dma_start(gam_sb[:], gam_d)
                bet_sb = constp.tile([128, QKF], F32, tag="bet")
                nc.sync.dma_start(bet_sb[:], bet_d)

            for b in range(B):
                # per-batch resident tiles (tags shared across batches)
                qt01 = resid.tile([128, S], F32R, tag="qt01")
                qt23 = resid.tile([128, S], F32R, tag="qt23")
                kt2 = resid.tile([128, S], F32R, tag="kt2")
                kt2b = resid.tile([128, S], F32R, tag="kt2b")
                vp = resid.tile([128, NBB, HD + 1], F32R, tag="vp")
                at01 = resid.tile([128, S], F32R, tag="at01")
                at23 = resid.tile([128, S], F32R, tag="at23")
                qtp = (qt01, qt23)
                atp = (at01, at23)

                nc.vector.tensor_copy(vp[:, :, HD : HD + 1], ones16[:].unsqueeze(2))
                nc.gpsimd.memset(kt2[64:128, :].bitcast(F32), 0.0)
                nc.gpsimd.memset(kt2b[0:64, :].bitcast(F32), 0.0)

                # ============ Phase A: QKV + LN + RoPE + transposes ============
                with (
                    tc.tile_pool(name="psA_xt", bufs=2, space="PSUM") as ps_xt,
                    tc.tile_pool(name="psA_qkv", bufs=2, space="PSUM") as ps_qkv,
                    tc.tile_pool(name="psA_qt", bufs=2, space="PSUM") as ps_qt,
                ):
                    for i in range(NBB):
                        row0 = b * S + i * 128
                        x_sb = work.tile([128, DIM], F32R, tag="x_sb")
                        nc.sync.dma_start(x_sb[:], x_d[row0 : row0 + 128, :])
                        # transpose x block -> xT tiles [d,tok], 4 per psum bank
                        xt_sb = work.tile([128, DIM // 128, 128], F32R, tag="xt_sb")
                        for c4 in range(4):
                            xt_ps = ps_xt.tile([128, 512], F32R, tag="xt_ps")
                            for j in range(4):
                                c = c4 * 4 + j
                                nc.tensor.transpose(
                                    xt_ps[:, j * 128 : (j + 1) * 128],
                                    x_sb[:, c * 128 : (c + 1) * 128],
                                    identr[:],
                                )
                            nc.scalar.copy(
                                xt_sb[:, c4 * 4 : c4 * 4 + 4, :]
                                .rearrange("p a b -> p (a b)"),
                                xt_ps[:],
                            )
                        # QKV projection: accumulate over d chunks
                        qkv_ps = ps_qkv.tile([128, FEAT], F32, tag="qkv_ps")
                        for c in range(DIM // 128):
                            nc.tensor.matmul(
                                qkv_ps[:],
                                xt_sb[:, c, :],
                                wt_sb[:, c, :],
                                start=(c == 0),
                                stop=(c == DIM // 128 - 1),
                            )
                        if dbg and b == 0 and i == 0:
                            dq = work.tile([128, FEAT], F32, tag="dbgq")
                            nc.scalar.copy(dq[:], qkv_ps[:])
                            nc.sync.dma_start(dbg_qkv, dq[:])
                        # copy q,k to sbuf fp32; v straight to V' (f32r)
                        qk = work.tile([128, QKF], F32, tag="qk")
                        nc.vector.tensor_copy(qk[:], qkv_ps[:, 0:QKF])
                        nc.vector.tensor_copy(
                            vp[:, i, 0:HD], qkv_ps[:, QKF:FEAT]
                        )
                        # LayerNorm over hd per head (5 heads: 4q + 1k)
                        st = work.tile([128, 5, 6], F32, tag="st")
                        mv = work.tile([128, 5, 2], F32, tag="mv")
                        for h in range(5):
                            nc.vector.bn_stats(
                                st[:, h, :], qk[:, h * HD : (h + 1) * HD]
                            )
                            nc.vector.bn_aggr(mv[:, h, :], st[:, h, :])
                        stdv = work.tile([128, 5], F32, tag="stdv")
                        nc.scalar.activation(
                            stdv[:], mv[:, :, 1], AF.Sqrt, bias=epsc[:]
                        )
                        rstd = work.tile([128, 5], F32, tag="rstd")
                        nc.vector.reciprocal_approx_fast(rstd[:], stdv[:])
                        qn = work.tile([128, QKF], F32, tag="qn")
                        for h in range(5):
                            sl = slice(h * HD, (h + 1) * HD)
                            nc.vector.tensor_scalar(
                                qn[:, sl],
                                qk[:, sl],
                                mv[:, h, 0:1],
                                rstd[:, h : h + 1],
                                op0=ALU.subtract,
                                op1=ALU.mult,
                            )
                        if use_gb:
                            nc.gpsimd.tensor_mul(qn[:], qn[:], gam_sb[:])
                            nc.gpsimd.tensor_add(qn[:], qn[:], bet_sb[:])
                        # RoPE
                        cosb = work.tile([128, QKF], F32, tag="cosb")
                        nc.sync.dma_start(cosb[:], cos_d[i * 128 : i * 128 + 128, :])
                        sinb = work.tile([128, QKF], F32, tag="sinb")
                        nc.sync.dma_start(sinb[:], sin_d[i * 128 : i * 128 + 128, :])
                        sw = work.tile([128, QKF], F32, tag="sw")
                        qn3 = qn[:].rearrange("p (a t) -> p a t", t=2)
                        sw3 = sw[:].rearrange("p (a t) -> p a t", t=2)
                        nc.vector.tensor_copy(sw3[:, :, 0], qn3[:, :, 1])
                        nc.vector.tensor_copy(sw3[:, :, 1], qn3[:, :, 0])
                        t1 = work.tile([128, QKF], F32, tag="t1")
                        nc.gpsimd.tensor_mul(t1[:], qn[:], cosb[:])
                        t2 = work.tile([128, QKF], F32, tag="t2")
                        nc.gpsimd.tensor_mul(t2[:], sw[:], sinb[:])
                        qr = work.tile([128, QKF], F32R, tag="qr")
                        nc.vector.tensor_add(qr[:], t1[:], t2[:])
                        if dbg and b == 0 and i == 0:
                            nc.sync.dma_start(dbg_qr, qr[:].bitcast(F32))
                        # transpose q heads + k to [hd, tok]; upper halves of
                        # the pair tiles are filled via SBUF->SBUF DMA (the
                        # only partition-crossing path).
                        for p in range(2):
                            for hh in range(2):
                                h = 2 * p + hh
                                qt_ps = ps_qt.tile(
                                    [64, 128], F32R, tag="qt_ps", bufs=4
                                )
                                nc.tensor.transpose(
                                    qt_ps[:],
                                    qr[:, h * HD : (h + 1) * HD],
                                    identr[:],
                                )
                                if hh == 0:
                                    if p == 0:
                                        nc.scalar.copy(
                                            qtp[p][0:64, i * 128 : (i + 1) * 128],
                                            qt_ps[:],
                                        )
                                    else:
                                        nc.vector.tensor_copy(
                                            qtp[p][0:64, i * 128 : (i + 1) * 128],
                                            qt_ps[:],
                                        )
                                else:
                                    stg_t = work3.tile(
                                        [64, 128], F32R, tag="stg_t"
                                    )
                                    if p == 0:
                                        nc.scalar.copy(stg_t[:], qt_ps[:])
                                    else:
                                        nc.vector.tensor_copy(stg_t[:], qt_ps[:])
                                    nc.sync.dma_start(
                                        qtp[p][64:128, i * 128 : (i + 1) * 128],
                                        stg_t[:],
                                    )
                        kt_ps = ps_qt.tile([64, 128], F32R, tag="qt_ps", bufs=4)
                        nc.tensor.transpose(kt_ps[:], qr[:, QF:QKF], identr[:])
                        nc.scalar.copy(
                            kt2[0:64, i * 128 : (i + 1) * 128], kt_ps[:]
                        )
                        nc.sync.dma_start(
                            kt2b[64:128, i * 128 : (i + 1) * 128],
                            kt2[0:64, i * 128 : (i + 1) * 128],
                        )

                if dbg and b == 0:
                    nc.sync.dma_start(dbg_qt01, qt01[:].bitcast(F32))
                    nc.sync.dma_start(dbg_kt2, kt2[:].bitcast(F32))
                    nc.sync.dma_start(
                        dbg_vp, vp[:].rearrange("p a b -> p (a b)").bitcast(F32)
                    )
                # ============ Phase B: attention ============
                with (
                    tc.tile_pool(name="psB_st", bufs=1, space="PSUM") as ps_st,
                    tc.tile_pool(name="psB_o", bufs=1, space="PSUM") as ps_o,
                ):
                    for p in range(2):
                        qt = qtp[p]
                        at = atp[p]
                        for qc in range(NQC):
                            kts = kt_list(qc, causal)
                            ot = [
                                [
                                    ps_o.tile(
                                        [65, 512], F32,
                                        tag=f"ot{h}{s}", name=f"ot{h}{s}",
                                    )
                                    for s in range(NSEG)
                                ]
                                for h in range(2)
                            ]
                            for kt in kts:
                                s0 = seg_lo(kt, qc, causal)
                                qs0 = s0 * 512
                                r = kt * 128 - qc * QCW
                                diag = causal and r >= 0
                                e0 = r if diag else qs0
                                pt = work.tile([128, 2, QCW], F32R, tag="pt")
                                if diag and e0 > qs0:
                                    # zero the fully-masked left columns
                                    nc.gpsimd.memset(
                                        pt[:, :, qs0:e0].bitcast(F32), 0.0
                                    )
                                for h in range(2):
                                    st_ps = ps_st.tile(
                                        [128, QCW], F32, tag=f"st{h}"
                                    )
                                    ktt = kt2 if h == 0 else kt2b
                                    if diag:
                                        # preload -1e9 above-diagonal into the
                                        # diag block; S^T accumulates on top
                                        nc.tensor.matmul(
                                            st_ps[:, r : r + 128],
                                            atri[:],
                                            negi[:],
                                            start=True,
                                            stop=False,
                                            skip_group_check=True,
                                        )
                                    for s in range(s0, NSEG):
                                        nc.tensor.matmul(
                                            st_ps[:, s * 512 : (s + 1) * 512],
                                            ktt[:, kt * 128 : (kt + 1) * 128],
                                            qt[:,
                                               qc * QCW + s * 512 : qc * QCW + (s + 1) * 512],
                                            start=not (diag and s == s0),
                                            stop=True,
                                            skip_group_check=True,
                                        )
                                    nc.scalar.activation(
                                        pt[:, h, e0:QCW],
                                        st_ps[:, e0:QCW],
                                        AF.Exp,
                                        scale=SCALE,
                                    )
                                if mask_mode == "general":
                                    emt = work.tile([128, QCW], F32R, tag="emt")
                                    nc.sync.dma_start(
                                        emt[:],
                                        emt_d[kt * 128 : (kt + 1) * 128,
                                              qc * QCW : (qc + 1) * QCW],
                                    )
                                    for h in range(2):
                                        nc.vector.tensor_mul(
                                            pt[:, h, :], pt[:, h, :], emt[:]
                                        )
                                if dbg and b == 0 and p == 0 and qc == 0 and kt == 0:
                                    nc.sync.dma_start(
                                        dbg_pt,
                                        pt[:].rearrange("p a b -> p (a b)").bitcast(F32),
                                    )
                                for s in range(s0, NSEG):
                                    last = (
                                        kt
                                        == min(
                                            kts[-1],
                                            (qc * QCW + (s + 1) * 512) // 128 - 1,
                                        )
                                        if causal
                                        else kt == kts[-1]
                                    )
                                    for h in range(2):
                                        nc.tensor.matmul(
                                            ot[h][s][:],
                                            vp[:, kt, :],
                                            pt[:, h, s * 512 : (s + 1) * 512],
                                            start=(kt == 0),
                                            stop=last,
                                        )
                            # normalize and write A^T
                            for h in range(2):
                                for s in range(NSEG):
                                    col0 = qc * QCW + s * 512
                                    den = work3.tile([1, 512], F32, tag="den")
                                    nc.vector.tensor_copy(
                                        den[:], ot[h][s][64:65, :]
                                    )
                                    rec = work3.tile([1, 512], F32, tag="rec")
                                    nc.vector.reciprocal_approx_fast(
                                        rec[:], den[:]
                                    )
                                    bcst = work3.tile([64, 512], F32, tag="bcst")
                                    nc.gpsimd.partition_broadcast(bcst[:], rec[:])
                                    if (dbg and b == 0 and p == 0 and qc == 0
                                            and h == 0 and s == 0):
                                        dot = work.tile([65, 512], F32, tag="dot")
                                        nc.scalar.copy(dot[:], ot[h][s][:])
                                        nc.sync.dma_start(dbg_ot, dot[:])
                                        nc.sync.dma_start(dbg_rec, rec[:])
                                        nc.sync.dma_start(dbg_bc, bcst[:])
                                    if h == 0:
                                        nc.vector.tensor_mul(
                                            at[0:64, col0 : col0 + 512],
                                            ot[h][s][0:64, :],
                                            bcst[:],
                                        )
                                    else:
                                        stg = work3.tile([64, 512], F32R, tag="stg")
                                        nc.vector.tensor_mul(
                                            stg[:], ot[h][s][0:64, :], bcst[:]
                                        )
                                        nc.sync.dma_start(
                                            at[64:128, col0 : col0 + 512], stg[:]
                                        )

                if dbg and b == 0:
                    nc.sync.dma_start(dbg_at01, at01[:].bitcast(F32))
                # ============ Phase C: output projection ============
                with tc.tile_pool(name="psC", bufs=3, space="PSUM") as ps_c:
                    for i in range(NBB):
                        row0 = b * S + i * 128
                        for dc in range(2):
                            o_ps = ps_c.tile([128, 1024], F32, tag="o_ps")
                            for dd in range(2):
                                for p in range(2):
                                    nc.tensor.matmul(
                                        o_ps[:, dd * 512 : (dd + 1) * 512],
                                        atp[p][:, i * 128 : (i + 1) * 128],
                                        wo_sb[:, p,
                                              (dc * 2 + dd) * 512 : (dc * 2 + dd + 1) * 512],
                                        start=(p == 0),
                                        stop=(p == 1),
                                    )
                            o_sb = work3.tile([128, 1024], F32, tag="o_sb")
                            if (i + dc) % 2 == 0:
                                nc.scalar.copy(o_sb[:], o_ps[:])
                            else:
                                nc.vector.tensor_copy(o_sb[:], o_ps[:])
                            nc.sync.dma_start(
                                out_d[row0 : row0 + 128,
                                      dc * 1024 : (dc + 1) * 1024],
                                o_sb[:],
                            )

    nc.compile()
    return nc


_CACHE = {}


def _get(mask_mode, use_gb):
    key = (mask_mode, use_gb)
    if key not in _CACHE:
        _CACHE[key] = build(mask_mode, use_gb)
    return _CACHE[key]


def _analyze_mask(mask):
    if not mask.any():
        return "none"
    neg = mask[0, 1]
    causal = np.where(
        np.tril(np.ones((S, S), dtype=bool)), np.float32(0), np.float32(neg)
    )
    if neg < -1e8 and np.array_equal(mask, causal):
        return "causal"
    return "general"


def prep_inputs(x, wqkv, wo, q_gamma, q_beta, k_gamma, k_beta,
                freqs_cos, freqs_sin, mask):
    mask_mode = _analyze_mask(np.asarray(mask))
    use_gb = not (
        np.all(q_gamma == 1.0) and np.all(k_gamma == 1.0)
        and np.all(q_beta == 0.0) and np.all(k_beta == 0.0)
    )
    x2 = np.ascontiguousarray(np.asarray(x, np.float32).reshape(TOK, DIM))
    cos_e = np.repeat(np.asarray(freqs_cos, np.float32), 2, axis=1)  # [S, 64]
    sin_e = np.repeat(np.asarray(freqs_sin, np.float32), 2, axis=1)
    sin_s = sin_e.copy()
    sin_s[:, 0::2] *= -1.0
    cos5 = np.ascontiguousarray(np.tile(cos_e, (1, 5)))
    sin5 = np.ascontiguousarray(np.tile(sin_s, (1, 5)))
    in_maps = []
    for g in range(NCORES):
        wq = wqkv[g * QF : (g + 1) * QF]
        wk = wqkv[NH * HD + g * HD : NH * HD + (g + 1) * HD]
        wv = wqkv[(NH + NKV) * HD + g * HD : (NH + NKV) * HD + (g + 1) * HD]
        wt = np.ascontiguousarray(
            np.concatenate([wq, wk, wv], axis=0).T.astype(np.float32)
        )
        wog = np.ascontiguousarray(wo[g * QF : (g + 1) * QF].astype(np.float32))
        m = {"x": x2, "wt": wt, "wo": wog, "cos5": cos5, "sin5": sin5}
        if use_gb:
            g5 = np.concatenate(
                [np.tile(np.asarray(q_gamma, np.float32), QH),
                 np.asarray(k_gamma, np.float32)]
            )
            b5 = np.concatenate(
                [np.tile(np.asarray(q_beta, np.float32), QH),
                 np.asarray(k_beta, np.float32)]
            )
            m["gam5"] = np.ascontiguousarray(np.broadcast_to(g5, (128, QKF)))
            m["bet5"] = np.ascontiguousarray(np.broadcast_to(b5, (128, QKF)))
        if mask_mode == "general":
            m["emt"] = np.ascontiguousarray(
                np.exp(np.asarray(mask, np.float32).T)
            )
        in_maps.append(m)
    return mask_mode, use_gb, in_maps


def kernel(x, wqkv, wo, q_gamma, q_beta, k_gamma, k_beta,
           freqs_cos, freqs_sin, mask, _trace=False):
    mask_mode, use_gb, in_maps = prep_inputs(
        x, wqkv, wo, q_gamma, q_beta, k_gamma, k_beta,
        freqs_cos, freqs_sin, mask,
    )
    nc = _get(mask_mode, use_gb)
    res = run_bass_kernel_spmd(
        nc, in_maps, core_ids=list(range(NCORES)), trace=_trace
    )
    out = res.results[0]["out"].astype(np.float64)
    for c in range(1, NCORES):
        out += res.results[c]["out"]
    kernel.last_result = res
    return out.astype(np.float32).reshape(B, S, DIM)


# revision 13
# speedup vs baseline: 1.3674x; 1.0172x over previous
"""Trainium2 Bass kernel for nn_Attention_56538949484622.

Full attention module (QKV proj + QK-LayerNorm + RoPE + GQA causal attention
+ output proj), tensor-parallel over heads across 8 NeuronCores.

Per-core shard g (of 8): q heads 4g..4g+3, kv head g, wqkv rows for those
heads, wo rows [256g:256(g+1)].  Each core computes a partial (B*S, DIM)
output; the host sums the 8 partials (the "all-reduce after wo").

Self-contained: hardcodes all shapes from the problem spec.
"""

import numpy as np

import concourse.bass as bass
from concourse import bacc
import concourse.mybir as mybir
from concourse.tile import TileContext
from concourse.bass_utils import run_bass_kernel_spmd
from concourse.masks import make_identity

F32 = mybir.dt.float32
F32R = mybir.dt.float32r
AF = mybir.ActivationFunctionType
ALU = mybir.AluOpType

B, S, DIM = 2, 2048, 2048
NH, NKV, HD = 32, 8, 64
NCORES = 8
QH = NH // NCORES            # 4 q heads per core
TOK = B * S                  # 4096
NBB = S // 128               # 16 token blocks per batch
QF = QH * HD                 # 256
QKF = QF + HD                # 320  (q heads + k head)
FEAT = QF + 2 * HD           # 384  (q + k + v)
EPS = 1e-5
SCALE = 1.0 / 8.0            # 1/sqrt(HD)
NQC = 2                      # q chunks of 1024 per batch
QCW = S // NQC               # 1024
NSEG = QCW // 512            # 512-wide segments per q chunk


def seg_lo(kt, qc, causal):
    """First valid 512-segment (within q chunk qc) for k tile kt."""
    if not causal:
        return 0
    return max(0, (kt * 128 - qc * QCW) // 512)


def kt_list(qc, causal):
    if not causal:
        return list(range(NBB))
    return list(range(min(NBB, (qc + 1) * QCW // 128)))


def build(mask_mode: str, use_gb: bool, dbg: bool = False):
    """mask_mode: 'causal' | 'none' | 'general'."""
    causal = mask_mode == "causal"
    nc = bacc.Bacc("TRN2", target_bir_lowering=False, debug=False)

    x_d = nc.dram_tensor("x", [TOK, DIM], F32R, kind="ExternalInput").ap()
    wt_d = nc.dram_tensor("wt", [DIM, FEAT], F32R, kind="ExternalInput").ap()
    wo_d = nc.dram_tensor("wo", [QF, DIM], F32R, kind="ExternalInput").ap()
    cos_d = nc.dram_tensor("cos5", [S, QKF], F32, kind="ExternalInput").ap()
    sin_d = nc.dram_tensor("sin5", [S, QKF], F32, kind="ExternalInput").ap()
    if use_gb:
        gam_d = nc.dram_tensor("gam5", [128, QKF], F32, kind="ExternalInput").ap()
        bet_d = nc.dram_tensor("bet5", [128, QKF], F32, kind="ExternalInput").ap()
    if mask_mode == "general":
        emt_d = nc.dram_tensor("emt", [S, S], F32R, kind="ExternalInput").ap()
    out_d = nc.dram_tensor("out", [TOK, DIM], F32, kind="ExternalOutput").ap()
    if dbg:
        dbg_qkv = nc.dram_tensor("dbg_qkv", [128, FEAT], F32, kind="ExternalOutput").ap()
        dbg_qr = nc.dram_tensor("dbg_qr", [128, QKF], F32, kind="ExternalOutput").ap()
        dbg_qt01 = nc.dram_tensor("dbg_qt01", [128, S], F32, kind="ExternalOutput").ap()
        dbg_kt2 = nc.dram_tensor("dbg_kt2", [128, S], F32, kind="ExternalOutput").ap()
        dbg_vp = nc.dram_tensor("dbg_vp", [128, NBB * (HD + 1)], F32, kind="ExternalOutput").ap()
        dbg_pt = nc.dram_tensor("dbg_pt", [128, 2 * QCW], F32, kind="ExternalOutput").ap()
        dbg_at01 = nc.dram_tensor("dbg_at01", [128, S], F32, kind="ExternalOutput").ap()
        dbg_ot = nc.dram_tensor("dbg_ot", [65, 512], F32, kind="ExternalOutput").ap()
        dbg_rec = nc.dram_tensor("dbg_rec", [1, 512], F32, kind="ExternalOutput").ap()
        dbg_bc = nc.dram_tensor("dbg_bc", [64, 512], F32, kind="ExternalOutput").ap()

    with TileContext(nc) as tc:
        with (
            tc.tile_pool(name="const", bufs=1) as constp,
            tc.tile_pool(name="resid", bufs=1) as resid,
            tc.tile_pool(name="work", bufs=2) as work,
            tc.tile_pool(name="work3", bufs=3) as work3,
        ):
            # ---- constants ----
            ident = constp.tile([128, 128], F32, tag="ident")
            make_identity(nc, ident[:])
            identr = constp.tile([128, 128], F32R, tag="identr")
            nc.vector.tensor_copy(identr[:], ident[:])
            ones16 = constp.tile([128, NBB], F32, tag="ones16")
            nc.vector.memset(ones16[:], 1.0)
            epsc = constp.tile([128, 1], F32, tag="epsc")
            nc.vector.memset(epsc[:], EPS)
            BF16 = mybir.dt.bfloat16
            atri = constp.tile([128, 128], BF16, tag="atri")
            nc.vector.memset(atri[:], 1.0)
            nc.gpsimd.affine_select(
                out=atri[:], in_=atri[:], compare_op=ALU.is_ge,
                fill=0.0, base=-1, channel_multiplier=-1, pattern=[[1, 128]],
            )
            negi = constp.tile([128, 128], BF16, tag="negi")
            nc.scalar.activation(negi[:], ident[:], AF.Copy, scale=-1.0e9)
            wt_sb = constp.tile([128, DIM // 128, FEAT], F32R, tag="wt")
            nc.sync.dma_start(
                wt_sb[:], wt_d.rearrange("(c p) f -> p c f", p=128)
            )
            wo_sb = constp.tile([128, 2, DIM], F32R, tag="wo")
            nc.sync.dma_start(
                wo_sb[:], wo_d.rearrange("(c p) d -> p c d", p=128)
            )
            if use_gb:
                gam_sb = constp.tile([128, QKF], F32, tag="gam")
                nc.sync.dma_start(gam_sb[:], gam_d)
                bet_sb = constp.tile([128, QKF], F32, tag="bet")
                nc.sync.dma_start(bet_sb[:], bet_d)

            for b in range(B):
                # per-batch resident tiles (tags shared across batches)
                qt01 = resid.tile([128, S], F32R, tag="qt01")
                qt23 = resid.tile([128, S], F32R, tag="qt23")
                kt2 = resid.tile([128, S], F32R, tag="kt2")
                kt2b = resid.tile([128, S], F32R, tag="kt2b")
                vp = resid.tile([128, NBB, HD + 1], F32R, tag="vp")
                at01 = resid.tile([128, S], F32R, tag="at01")
                at23 = resid.tile([128, S], F32R, tag="at23")
                qtp = (qt01, qt23)
                atp = (at01, at23)

                nc.vector.tensor_copy(vp[:, :, HD : HD + 1], ones16[:].unsqueeze(2))
                nc.gpsimd.memset(kt2[64:128, :].bitcast(F32), 0.0)
                nc.gpsimd.memset(kt2b[0:64, :].bitcast(F32), 0.0)

                # ============ Phase A: QKV + LN + RoPE + transposes ============
                with (
                    tc.tile_pool(name="psA_xt", bufs=2, space="PSUM") as ps_xt,
                    tc.tile_pool(name="psA_qkv", bufs=2, space="PSUM") as ps_qkv,
                    tc.tile_pool(name="psA_qt", bufs=2, space="PSUM") as ps_qt,
                ):
                    for i in range(NBB):
                        row0 = b * S + i * 128
                        x_sb = work.tile([128, DIM], F32R, tag="x_sb")
                        nc.sync.dma_start(x_sb[:], x_d[row0 : row0 + 128, :])
                        # transpose x block -> xT tiles [d,tok], 4 per psum bank
                        xt_sb = work.tile([128, DIM // 128, 128], F32R, tag="xt_sb")
                        for c4 in range(4):
                            xt_ps = ps_xt.tile([128, 512], F32R, tag="xt_ps")
                            for j in range(4):
                                c = c4 * 4 + j
                                nc.tensor.transpose(
                                    xt_ps[:, j * 128 : (j + 1) * 128],
                                    x_sb[:, c * 128 : (c + 1) * 128],
                                    identr[:],
                                )
                            nc.scalar.copy(
                                xt_sb[:, c4 * 4 : c4 * 4 + 4, :]
                                .rearrange("p a b -> p (a b)"),
                                xt_ps[:],
                            )
                        # QKV projection: accumulate over d chunks
                        qkv_ps = ps_qkv.tile([128, FEAT], F32, tag="qkv_ps")
                        for c in range(DIM // 128):
                            nc.tensor.matmul(
                                qkv_ps[:],
                                xt_sb[:, c, :],
                                wt_sb[:, c, :],
                                start=(c == 0),
                                stop=(c == DIM // 128 - 1),
                            )
                        if dbg and b == 0 and i == 0:
                            dq = work.tile([128, FEAT], F32, tag="dbgq")
                            nc.scalar.copy(dq[:], qkv_ps[:])
                            nc.sync.dma_start(dbg_qkv, dq[:])
                        # copy q,k to sbuf fp32; v straight to V' (f32r)
                        qk = work.tile([128, QKF], F32, tag="qk")
                        nc.vector.tensor_copy(qk[:], qkv_ps[:, 0:QKF])
                        nc.vector.tensor_copy(
                            vp[:, i, 0:HD], qkv_ps[:, QKF:FEAT]
                        )
                        # LayerNorm over hd per head (5 heads: 4q + 1k)
                        st = work.tile([128, 5, 6], F32, tag="st")
                        mv = work.tile([128, 5, 2], F32, tag="mv")
                        for h in range(5):
                            nc.vector.bn_stats(
                                st[:, h, :], qk[:, h * HD : (h + 1) * HD]
                            )
                            nc.vector.bn_aggr(mv[:, h, :], st[:, h, :])
                        stdv = work.tile([128, 5], F32, tag="stdv")
                        nc.scalar.activation(
                            stdv[:], mv[:, :, 1], AF.Sqrt, bias=epsc[:]
                        )
                        rstd = work.tile([128, 5], F32, tag="rstd")
                        nc.vector.reciprocal_approx_fast(rstd[:], stdv[:])
                        qn = work.tile([128, QKF], F32, tag="qn")
                        for h in range(5):
                            sl = slice(h * HD, (h + 1) * HD)
                            nc.vector.tensor_scalar(
                                qn[:, sl],
                                qk[:, sl],
                                mv[:, h, 0:1],
                                rstd[:, h : h + 1],
                                op0=ALU.subtract,
                                op1=ALU.mult,
                            )
                        if use_gb:
                            nc.gpsimd.tensor_mul(qn[:], qn[:], gam_sb[:])
                            nc.gpsimd.tensor_add(qn[:], qn[:], bet_sb[:])
                        # RoPE
                        cosb = work.tile([128, QKF], F32, tag="cosb")
                        nc.sync.dma_start(cosb[:], cos_d[i * 128 : i * 128 + 128, :])
                        sinb = work.tile([128, QKF], F32, tag="sinb")
                        nc.sync.dma_start(sinb[:], sin_d[i * 128 : i * 128 + 128, :])
                        sw = work.tile([128, QKF], F32, tag="sw")
                        qn3 = qn[:].rearrange("p (a t) -> p a t", t=2)
                        sw3 = sw[:].rearrange("p (a t) -> p a t", t=2)
                        nc.vector.tensor_copy(sw3[:, :, 0], qn3[:, :, 1])
                        nc.vector.tensor_copy(sw3[:, :, 1], qn3[:, :, 0])
                        t1 = work.tile([128, QKF], F32, tag="t1")
                        nc.gpsimd.tensor_mul(t1[:], qn[:], cosb[:])
                        t2 = work.tile([128, QKF], F32, tag="t2")
                        nc.gpsimd.tensor_mul(t2[:], sw[:], sinb[:])
                        qr = work.tile([128, QKF], F32R, tag="qr")
                        nc.vector.tensor_add(qr[:], t1[:], t2[:])
                        if dbg and b == 0 and i == 0:
                            nc.sync.dma_start(dbg_qr, qr[:].bitcast(F32))
                        # transpose q heads + k to [hd, tok]; upper halves of
                        # the pair tiles are filled via SBUF->SBUF DMA (the
                        # only partition-crossing path).
                        for p in range(2):
                            for hh in range(2):
                                h = 2 * p + hh
                                qt_ps = ps_qt.tile(
                                    [64, 128], F32R, tag="qt_ps", bufs=4
                                )
                                nc.tensor.transpose(
                                    qt_ps[:],
                                    qr[:, h * HD : (h + 1) * HD],
                                    identr[:],
                                )
                                if hh == 0:
                                    if p == 0:
                                        nc.scalar.copy(
                                            qtp[p][0:64, i * 128 : (i + 1) * 128],
                                            qt_ps[:],
                                        )
                                    else:
                                        nc.vector.tensor_copy(
                                            qtp[p][0:64, i * 128 : (i + 1) * 128],
                                            qt_ps[:],
                                        )
                                else:
                                    stg_t = work3.tile(
                                        [64, 128], F32R, tag="stg_t"
                                    )
                                    if p == 0:
                                        nc.scalar.copy(stg_t[:], qt_ps[:])
                                    else:
                                        nc.vector.tensor_copy(stg_t[:], qt_ps[:])
                                    nc.sync.dma_start(
                                        qtp[p][64:128, i * 128 : (i + 1) * 128],
                                        stg_t[:],
                                    )
                        kt_ps = ps_qt.tile([64, 128], F32R, tag="qt_ps", bufs=4)
                        nc.tensor.transpose(kt_ps[:], qr[:, QF:QKF], identr[:])
                        nc.scalar.copy(
                            kt2[0:64, i * 128 : (i + 1) * 128], kt_ps[:]
                        )
                        nc.sync.dma_start(
                            kt2b[64:128, i * 128 : (i + 1) * 128],
                            kt2[0:64, i * 128 : (i + 1) * 128],
                        )

                if dbg and b == 0:
                    nc.sync.dma_start(dbg_qt01, qt01[:].bitcast(F32))
                    nc.sync.dma_start(dbg_kt2, kt2[:].bitcast(F32))
                    nc.sync.dma_start(
                        dbg_vp, vp[:].rearrange("p a b -> p (a b)").bitcast(F32)
                    )
                # ============ Phase B: attention ============
                with (
                    tc.tile_pool(name="psB_st", bufs=1, space="PSUM") as ps_st,
                    tc.tile_pool(name="psB_o", bufs=1, space="PSUM") as ps_o,
                ):
                    for p in range(2):
                        qt = qtp[p]
                        at = atp[p]
                        for qc in range(NQC):
                            kts = kt_list(qc, causal)
                            ot = [
                                [
                                    ps_o.tile(
                                        [65, 512], F32,
                                        tag=f"ot{h}{s}", name=f"ot{h}{s}",
                                    )
                                    for s in range(NSEG)
                                ]
                                for h in range(2)
                            ]
                            for kt in kts:
                                s0 = seg_lo(kt, qc, causal)
                                qs0 = s0 * 512
                                r = kt * 128 - qc * QCW
                                diag = causal and r >= 0
                                e0 = r if diag else qs0
                                pt = work.tile([128, 2, QCW], F32R, tag="pt", bufs=3)
                                if diag and e0 > qs0:
                                    # zero the fully-masked left columns
                                    nc.gpsimd.memset(
                                        pt[:, :, qs0:e0].bitcast(F32), 0.0
                                    )
                                for h in range(2):
                                    st_ps = ps_st.tile(
                                        [128, QCW], F32, tag=f"st{h}"
                                    )
                                    ktt = kt2 if h == 0 else kt2b
                                    if diag:
                                        # preload -1e9 above-diagonal into the
                                        # diag block; S^T accumulates on top
                                        nc.tensor.matmul(
                                            st_ps[:, r : r + 128],
                                            atri[:],
                                            negi[:],
                                            start=True,
                                            stop=False,
                                            skip_group_check=True,
                                        )
                                    for s in range(s0, NSEG):
                                        nc.tensor.matmul(
                                            st_ps[:, s * 512 : (s + 1) * 512],
                                            ktt[:, kt * 128 : (kt + 1) * 128],
                                            qt[:,
                                               qc * QCW + s * 512 : qc * QCW + (s + 1) * 512],
                                            start=not (diag and s == s0),
                                            stop=True,
                                            skip_group_check=True,
                                        )
                                    nc.scalar.activation(
                                        pt[:, h, e0:QCW],
                                        st_ps[:, e0:QCW],
                                        AF.Exp,
                                        scale=SCALE,
                                    )
                                if mask_mode == "general":
                                    emt = work.tile([128, QCW], F32R, tag="emt")
                                    nc.sync.dma_start(
                                        emt[:],
                                        emt_d[kt * 128 : (kt + 1) * 128,
                                              qc * QCW : (qc + 1) * QCW],
                                    )
                                    for h in range(2):
                                        nc.vector.tensor_mul(
                                            pt[:, h, :], pt[:, h, :], emt[:]
                                        )
                                if dbg and b == 0 and p == 0 and qc == 0 and kt == 0:
                                    nc.sync.dma_start(
                                        dbg_pt,
                                        pt[:].rearrange("p a b -> p (a b)").bitcast(F32),
                                    )
                                for s in range(s0, NSEG):
                                    last = (
                                        kt
                                        == min(
                                            kts[-1],
                                            (qc * QCW + (s + 1) * 512) // 128 - 1,
                                        )
                                        if causal
                                        else kt == kts[-1]
                                    )
                                    for h in range(2):
                                        nc.tensor.matmul(
                                            ot[h][s][:],
                                            vp[:, kt, :],
                                            pt[:, h, s * 512 : (s + 1) * 512],
                                            start=(kt == 0),
                                            stop=last,
                                        )
                            # normalize and write A^T
                            for h in range(2):
                                for s in range(NSEG):
                                    col0 = qc * QCW + s * 512
                                    den = work3.tile([1, 512], F32, tag="den")
                                    nc.vector.tensor_copy(
                                        den[:], ot[h][s][64:65, :]
                                    )
                                    rec = work3.tile([1, 512], F32, tag="rec")
                                    nc.vector.reciprocal_approx_fast(
                                        rec[:], den[:]
                                    )
                                    bcst = work3.tile([64, 512], F32, tag="bcst")
                                    nc.gpsimd.partition_broadcast(bcst[:], rec[:])
                                    if (dbg and b == 0 and p == 0 and qc == 0
                                            and h == 0 and s == 0):
                                        dot = work.tile([65, 512], F32, tag="dot")
                                        nc.scalar.copy(dot[:], ot[h][s][:])
                                        nc.sync.dma_start(dbg_ot, dot[:])
                                        nc.sync.dma_start(dbg_rec, rec[:])
                                        nc.sync.dma_start(dbg_bc, bcst[:])
                                    if h == 0:
                                        nc.vector.tensor_mul(
                                            at[0:64, col0 : col0 + 512],
                                            ot[h][s][0:64, :],
                                            bcst[:],
                                        )
                                    else:
                                        stg = work3.tile([64, 512], F32R, tag="stg")
                                        nc.vector.tensor_mul(
                                            stg[:], ot[h][s][0:64, :], bcst[:]
                                        )
                                        nc.sync.dma_start(
                                            at[64:128, col0 : col0 + 512], stg[:]
                                        )

                if dbg and b == 0:
                    nc.sync.dma_start(dbg_at01, at01[:].bitcast(F32))
                # ============ Phase C: output projection ============
                with tc.tile_pool(name="psC", bufs=3, space="PSUM") as ps_c:
                    for i in range(NBB):
                        row0 = b * S + i * 128
                        for dc in range(2):
                            o_ps = ps_c.tile([128, 1024], F32, tag="o_ps")
                            for dd in range(2):
                                for p in range(2):
                                    nc.tensor.matmul(
                                        o_ps[:, dd * 512 : (dd + 1) * 512],
                                        atp[p][:, i * 128 : (i + 1) * 128],
                                        wo_sb[:, p,
                                              (dc * 2 + dd) * 512 : (dc * 2 + dd + 1) * 512],
                                        start=(p == 0),
                                        stop=(p == 1),
                                    )
                            o_sb = work3.tile([128, 1024], F32, tag="o_sb")
                            if (i + dc) % 2 == 0:
                                nc.scalar.copy(o_sb[:], o_ps[:])
                            else:
                                nc.vector.tensor_copy(o_sb[:], o_ps[:])
                            nc.sync.dma_start(
                                out_d[row0 : row0 + 128,
                                      dc * 1024 : (dc + 1) * 1024],
                                o_sb[:],
                            )

    nc.compile()
    return nc


_CACHE = {}


def _get(mask_mode, use_gb):
    key = (mask_mode, use_gb)
    if key not in _CACHE:
        _CACHE[key] = build(mask_mode, use_gb)
    return _CACHE[key]


def _analyze_mask(mask):
    if not mask.any():
        return "none"
    neg = mask[0, 1]
    causal = np.where(
        np.tril(np.ones((S, S), dtype=bool)), np.float32(0), np.float32(neg)
    )
    if neg < -1e8 and np.array_equal(mask, causal):
        return "causal"
    return "general"


def prep_inputs(x, wqkv, wo, q_gamma, q_beta, k_gamma, k_beta,
                freqs_cos, freqs_sin, mask):
    mask_mode = _analyze_mask(np.asarray(mask))
    use_gb = not (
        np.all(q_gamma == 1.0) and np.all(k_gamma == 1.0)
        and np.all(q_beta == 0.0) and np.all(k_beta == 0.0)
    )
    x2 = np.ascontiguousarray(np.asarray(x, np.float32).reshape(TOK, DIM))
    cos_e = np.repeat(np.asarray(freqs_cos, np.float32), 2, axis=1)  # [S, 64]
    sin_e = np.repeat(np.asarray(freqs_sin, np.float32), 2, axis=1)
    sin_s = sin_e.copy()
    sin_s[:, 0::2] *= -1.0
    cos5 = np.ascontiguousarray(np.tile(cos_e, (1, 5)))
    sin5 = np.ascontiguousarray(np.tile(sin_s, (1, 5)))
    in_maps = []
    for g in range(NCORES):
        wq = wqkv[g * QF : (g + 1) * QF]
        wk = wqkv[NH * HD + g * HD : NH * HD + (g + 1) * HD]
        wv = wqkv[(NH + NKV) * HD + g * HD : (NH + NKV) * HD + (g + 1) * HD]
        wt = np.ascontiguousarray(
            np.concatenate([wq, wk, wv], axis=0).T.astype(np.float32)
        )
        wog = np.ascontiguousarray(wo[g * QF : (g + 1) * QF].astype(np.float32))
        m = {"x": x2, "wt": wt, "wo": wog, "cos5": cos5, "sin5": sin5}
        if use_gb:
            g5 = np.concatenate(
                [np.tile(np.asarray(q_gamma, np.float32), QH),
                 np.asarray(k_gamma, np.float32)]
            )
            b5 = np.concatenate(
                [np.tile(np.asarray(q_beta, np.float32), QH),
                 np.asarray(k_beta, np.float32)]
            )
            m["gam5"] = np.ascontiguousarray(np.broadcast_to(g5, (128, QKF)))
            m["bet5"] = np.ascontiguousarray(np.broadcast_to(b5, (128, QKF)))
        if mask_mode == "general":
            m["emt"] = np.ascontiguousarray(
                np.exp(np.asarray(mask, np.float32).T)
            )
        in_maps.append(m)
    return mask_mode, use_gb, in_maps


def kernel(x, wqkv, wo, q_gamma, q_beta, k_gamma, k_beta,
           freqs_cos, freqs_sin, mask, _trace=False):
    mask_mode, use_gb, in_maps = prep_inputs(
        x, wqkv, wo, q_gamma, q_beta, k_gamma, k_beta,
        freqs_cos, freqs_sin, mask,
    )
    nc = _get(mask_mode, use_gb)
    res = run_bass_kernel_spmd(
        nc, in_maps, core_ids=list(range(NCORES)), trace=_trace
    )
    out = res.results[0]["out"].astype(np.float64)
    for c in range(1, NCORES):
        out += res.results[c]["out"]
    kernel.last_result = res
    return out.astype(np.float32).reshape(B, S, DIM)
